# revision 8
# baseline (speedup 1.0000x reference)
"""Causal multi-head attention with RoPE on 8 Trainium2 NeuronCores.

Sharding: core = (batch b, head-group hg): b = core//4, hg = core%4.
Each core computes 4 heads of one batch element end-to-end (QKV projection,
RoPE, causal softmax attention, output-projection partial) and the host sums
the 4 per-head-group partials per batch (the "all-reduce" of the O-proj).

Device-side layout choices (per core):
  xT   [1024, 2048]  x[b] transposed (d on partitions, 8 chunks of 128)
  q^T,k^T  [256, 2048] head-transposed, computed with W^T stationary
  v    [l, e] layout with a ones column appended per head (denominator trick)
  scores computed transposed: S^T[k, q] = k^T.T @ q^T, exp with no max
  subtraction (scores are bounded ~17; exp fits fp32 easily), causal mask
  applied multiplicatively post-exp, AV matmul gives O^T[e+1, q] whose last
  row is the softmax denominator. Normalization via a rank-1 broadcast matmul.
Matmuls run in float32r (TF32): full PE rate for moving dims >= 256. The
verifier requires fp32r matmul inputs to be *produced* rounded, so all
matmul-feeding tensors are declared float32r (DMA'd ones pre-rounded on host,
engine-written ones rounded on write).
"""

import numpy as np

_B, _L, _D, _H, _HD = 2, 2048, 1024, 16, 64
_HPG = 4              # heads per group (per core)
_EG = _HPG * _HD      # 256
_NCORES = 8
_THETA = 10000.0
_QC = 512             # q-chunk width
_NQC = _L // _QC      # 4
_GK = 2               # k-tiles (128) per exp group
_NKC = _D // 128      # 8 contraction chunks for projections
_LC = 512             # l-chunk for phase 1
_NLC = _L // _LC

_CACHE = {}


def _tf32(a):
    """Round float32 array to TF32 (fp32r): RNE to 10-bit mantissa."""
    b = np.ascontiguousarray(a, dtype=np.float32).view(np.uint32)
    b = (b + np.uint32(0xFFF) + ((b >> np.uint32(13)) & np.uint32(1))) \
        & np.uint32(0xFFFFE000)
    return b.view(np.float32)


def _build_nc():
    from contextlib import ExitStack

    import concourse.mybir as mybir
    import concourse.tile as tile
    from concourse import bacc

    f32 = mybir.dt.float32
    f32r = mybir.dt.float32r
    EXP = mybir.ActivationFunctionType.Exp

    nc = bacc.Bacc("TRN2", target_bir_lowering=False, debug=False,
                   enable_asserts=False)
    xT = nc.dram_tensor("xT", [_D, _L], f32r, kind="ExternalInput")
    wq = nc.dram_tensor("wq", [_D, _EG], f32r, kind="ExternalInput")
    wk = nc.dram_tensor("wk", [_D, _EG], f32r, kind="ExternalInput")
    wv = nc.dram_tensor("wv", [_D, _EG], f32r, kind="ExternalInput")
    wo = nc.dram_tensor("wo", [_EG, _D], f32r, kind="ExternalInput")
    cs = nc.dram_tensor("cs", [128, _L], f32, kind="ExternalInput")
    sn = nc.dram_tensor("sn", [128, _L], f32, kind="ExternalInput")
    msk = nc.dram_tensor("msk", [128, _QC // 128, _QC], f32,
                         kind="ExternalInput")
    perm = nc.dram_tensor("perm", [128, 128], f32r, kind="ExternalInput")
    vones = nc.dram_tensor("vones", [128, _HD], f32r, kind="ExternalInput")
    y = nc.dram_tensor("y", [_L, _D], f32, kind="ExternalOutput")

    with tile.TileContext(nc) as tc, ExitStack() as ctx:
        persist = ctx.enter_context(tc.tile_pool(name="persist", bufs=1))
        qT_sb = persist.tile([128, 2, _L], f32r)
        kT_sb = persist.tile([128, 2, _L], f32r)
        v_sb = persist.tile([128, _L // 128, _HPG, _HD + 1], f32r)
        oT_sb = persist.tile([128, 2, _L], f32r)
        wo_sb = persist.tile([128, 2, _D], f32r)
        msk_sb = persist.tile([128, _QC // 128, _QC], f32)
        ones_sb = persist.tile([128, _HD], f32r)

        nc.sync.dma_start(out=wo_sb, in_=wo.rearrange("(c p) d -> p c d", p=128))
        nc.sync.dma_start(out=msk_sb, in_=msk[:, :, :])
        nc.sync.dma_start(out=ones_sb, in_=vones[:, :])
        nc.sync.dma_start(
            out=v_sb[:, :, :, _HD:],
            in_=vones.rearrange("p (a b) -> p a b",
                                a=_L // 128).unsqueeze(3))

        # ---- Phase 1: projections + RoPE, streamed over l-chunks ----
        with tc.tile_pool(name="p1", bufs=1) as p1, \
             tc.tile_pool(name="xtp", bufs=2) as xtp, \
             tc.tile_pool(name="p1ps", bufs=3, space="PSUM") as p1ps, \
             tc.tile_pool(name="vps", bufs=2, space="PSUM") as vps, \
             tc.tile_pool(name="rotp", bufs=2, space="PSUM") as rotp, \
             tc.tile_pool(name="rtmp", bufs=3) as rtmp:
            wq_sb = p1.tile([128, _NKC, _EG], f32r)
            wk_sb = p1.tile([128, _NKC, _EG], f32r)
            wv_sb = p1.tile([128, _NKC, _EG], f32r)
            cs_sb = p1.tile([128, _L], f32)
            sn_sb = p1.tile([128, _L], f32)
            perm_sb = p1.tile([128, 128], f32r)
            nc.sync.dma_start(out=wq_sb, in_=wq.rearrange("(c p) e -> p c e", p=128))
            nc.sync.dma_start(out=wk_sb, in_=wk.rearrange("(c p) e -> p c e", p=128))
            nc.sync.dma_start(out=wv_sb, in_=wv.rearrange("(c p) e -> p c e", p=128))
            nc.sync.dma_start(out=cs_sb, in_=cs[:, :])
            nc.sync.dma_start(out=sn_sb, in_=sn[:, :])
            nc.sync.dma_start(out=perm_sb, in_=perm[:, :])

            xT_r = xT.rearrange("(c p) l -> p c l", p=128)
            for lc in range(_NLC):
                ls = slice(lc * _LC, (lc + 1) * _LC)
                xt = xtp.tile([128, _NKC, _LC], f32r, tag="xt")
                nc.sync.dma_start(out=xt, in_=xT_r[:, :, ls])
                for w_sb, dst in ((wq_sb, qT_sb), (wk_sb, kT_sb)):
                    for c in range(2):
                        ps = p1ps.tile([128, _LC], f32, tag="proj")
                        for kc in range(_NKC):
                            nc.tensor.matmul(
                                ps, w_sb[:, kc, c * 128:(c + 1) * 128],
                                xt[:, kc, :],
                                start=(kc == 0), stop=(kc == _NKC - 1))
                        nc.scalar.copy(dst[:, c, ls], ps)
                for j in range(_LC // 128):
                    lt = lc * (_LC // 128) + j
                    pv = vps.tile([128, _EG], f32, tag="vp")
                    for kc in range(_NKC):
                        nc.tensor.matmul(
                            pv, xt[:, kc, j * 128:(j + 1) * 128],
                            wv_sb[:, kc, :],
                            start=(kc == 0), stop=(kc == _NKC - 1))
                    nc.vector.tensor_copy(
                        v_sb[:, lt, :, :_HD],
                        pv.rearrange("p (h e) -> p h e", h=_HPG))
                # RoPE (in place on this l-chunk): x*cos + (perm@x)*sin
                for dst in (qT_sb, kT_sb):
                    for c in range(2):
                        rp = rotp.tile([128, _LC], f32, tag="rot")
                        nc.tensor.matmul(rp, perm_sb[:, :], dst[:, c, ls],
                                         start=True, stop=True)
                        tmp = rtmp.tile([128, _LC], f32, tag="rt")
                        nc.vector.tensor_mul(tmp, rp, sn_sb[:, ls])
                        nc.vector.tensor_mul(dst[:, c, ls], dst[:, c, ls],
                                             cs_sb[:, ls])
                        nc.vector.tensor_add(dst[:, c, ls], dst[:, c, ls], tmp)

        # ---- Phase 2: attention + output projection ----
        with tc.tile_pool(name="ptp", bufs=8) as ptp, \
             tc.tile_pool(name="sps", bufs=2, space="PSUM") as sps, \
             tc.tile_pool(name="ops", bufs=2, space="PSUM") as ops, \
             tc.tile_pool(name="bps", bufs=1, space="PSUM") as bps, \
             tc.tile_pool(name="opps", bufs=1, space="PSUM") as opps, \
             tc.tile_pool(name="rcp", bufs=2) as rcp, \
             tc.tile_pool(name="otn", bufs=3) as otnp, \
             tc.tile_pool(name="outp", bufs=4) as outp:
            for qc in range(_NQC):
                q0 = qc * _QC
                qs = slice(q0, q0 + _QC)
                nkt = (qc + 1) * (_QC // 128)
                for h in range(_HPG):
                    c, pb = h // 2, 64 * (h % 2)
                    ot = ops.tile([_HD + 1, _QC], f32, tag="ot")
                    ngr = (nkt + _GK - 1) // _GK
                    for g in range(ngr):
                        kts = list(range(g * _GK, min((g + 1) * _GK, nkt)))
                        sp = sps.tile([128, _GK * _QC], f32, tag="sp")
                        for i, kt in enumerate(kts):
                            nc.tensor.matmul(
                                sp[:, i * _QC:(i + 1) * _QC],
                                kT_sb[pb:pb + 64, c, kt * 128:(kt + 1) * 128],
                                qT_sb[pb:pb + 64, c, qs],
                                start=True, stop=True)
                        pt = ptp.tile([128, _GK * _QC], f32r, tag="pt")
                        na = len(kts) * _QC
                        nc.scalar.activation(pt[:, :na], sp[:, :na], EXP,
                                             scale=0.125)
                        for i, kt in enumerate(kts):
                            dj = kt - qc * (_QC // 128)
                            if dj >= 0:
                                nc.vector.tensor_mul(
                                    pt[:, i * _QC:(i + 1) * _QC],
                                    pt[:, i * _QC:(i + 1) * _QC],
                                    msk_sb[:, dj, :])
                        for i, kt in enumerate(kts):
                            nc.tensor.matmul(
                                ot, v_sb[:, kt, h, :],
                                pt[:, i * _QC:(i + 1) * _QC],
                                start=(kt == 0), stop=(kt == nkt - 1),
                                skip_group_check=True)
                    # normalize: divide rows 0..63 by row 64 (denominator)
                    rec = rcp.tile([128, _QC], f32, tag="rec")
                    nc.vector.reciprocal(rec[64:65, :], ot[64:65, :])
                    rec_r = rcp.tile([128, _QC], f32r, tag="rec_r")
                    nc.scalar.copy(rec_r[64:65, :], rec[64:65, :])
                    bc = bps.tile([64, _QC], f32, tag="bc")
                    nc.tensor.matmul(bc, ones_sb[64:65, :], rec_r[64:65, :],
                                     start=True, stop=True)
                    bcs = otnp.tile([64, _QC], f32, tag="bcs")
                    nc.scalar.copy(bcs, bc[:, :])
                    otn = otnp.tile([64, _QC], f32r, tag="otn")
                    nc.vector.tensor_mul(otn, ot[0:64, :], bcs)
                    # place normalized O^T at this head's partitions (DMA can
                    # cross partition bases; compute engines cannot)
                    nc.sync.dma_start(out=oT_sb[pb:pb + 64, c, qs], in_=otn)
                # output projection for this q-range
                for j in range(_QC // 128):
                    lt = qc * (_QC // 128) + j
                    for n in range(2):
                        op = opps.tile([128, 512], f32, tag="op")
                        for cc in range(2):
                            nc.tensor.matmul(
                                op, oT_sb[:, cc, lt * 128:(lt + 1) * 128],
                                wo_sb[:, cc, n * 512:(n + 1) * 512],
                                start=(cc == 0), stop=(cc == 1))
                        ob = outp.tile([128, 512], f32, tag="ob")
                        nc.vector.tensor_copy(ob, op)
                        nc.sync.dma_start(
                            out=y[lt * 128:(lt + 1) * 128,
                                  n * 512:(n + 1) * 512],
                            in_=ob)
    nc.compile()
    return nc


def get_nc():
    if "nc" not in _CACHE:
        _CACHE["nc"] = _build_nc()
    return _CACHE["nc"]


def make_in_maps(x, token_positions, Q, K, V, O_w):
    """Host-side sharding: per-core input dict (core = b*4 + hg)."""
    x = np.asarray(x, dtype=np.float32)
    tp = np.asarray(token_positions)
    Q = np.asarray(Q, dtype=np.float32)
    K = np.asarray(K, dtype=np.float32)
    V = np.asarray(V, dtype=np.float32)
    O_w = np.asarray(O_w, dtype=np.float32)

    # RoPE tables, [128, L]: rows 0..63 head-local e (cos repeated pairwise),
    # rows 64..127 a copy (two heads share one partition tile).
    i = np.arange(_HD // 2, dtype=np.float64)
    denom = _THETA ** (2.0 * i / _HD)                      # [32]
    ang = tp.astype(np.float64)[None, :] / denom[:, None]  # [32, L]
    cs64 = np.repeat(np.cos(ang), 2, axis=0)
    sn64 = np.repeat(np.sin(ang), 2, axis=0)
    cs = np.vstack([cs64, cs64]).astype(np.float32)
    sn = np.vstack([sn64, sn64]).astype(np.float32)

    # pairwise-rotation permutation (rot(x)[2i] = -x[2i+1], rot[2i+1] = x[2i])
    # as a stationary operand: out = permT.T @ x^T = Perm @ x^T
    p64 = np.zeros((64, 64), np.float32)
    for j in range(_HD // 2):
        p64[2 * j + 1, 2 * j] = -1.0
        p64[2 * j, 2 * j + 1] = 1.0
    permT = np.zeros((128, 128), np.float32)
    permT[0:64, 0:64] = p64
    permT[64:128, 64:128] = p64

    # causal masks for the 4 diagonal k-tiles of a 512-wide q-chunk
    pp = np.arange(128)[:, None]
    ff = np.arange(_QC)[None, :]
    msk = np.stack([(ff >= 128 * j + pp) for j in range(_QC // 128)],
                   axis=1).astype(np.float32)             # [128, 4, 512]

    Qr = Q.reshape(_H, _HD, _D)
    Kr = K.reshape(_H, _HD, _D)
    Vr = V.reshape(_H, _HD, _D)

    in_maps = []
    xT = [_tf32(x[b].T) for b in range(_B)]
    for core in range(_NCORES):
        b, hg = core // 4, core % 4
        hs = slice(hg * _HPG, (hg + 1) * _HPG)
        in_maps.append({
            "xT": xT[b],
            "wq": _tf32(Qr[hs].reshape(_EG, _D).T),
            "wk": _tf32(Kr[hs].reshape(_EG, _D).T),
            "wv": _tf32(Vr[hs].reshape(_EG, _D).T),
            "wo": _tf32(O_w[:, hg * _EG:(hg + 1) * _EG].T),
            "cs": cs, "sn": sn, "msk": msk, "perm": permT,
            "vones": np.ones((128, _HD), np.float32),
        })
    return in_maps


def run_on_hw(in_maps, trace=False, **kw):
    from concourse.bass_utils import run_bass_kernel_spmd
    nc = get_nc()
    return run_bass_kernel_spmd(nc, in_maps, core_ids=list(range(_NCORES)),
                                trace=trace, **kw)


def kernel(x, token_positions, Q, K, V, O_w):
    in_maps = make_in_maps(x, token_positions, Q, K, V, O_w)
    res = run_on_hw(in_maps)
    out = np.zeros((_B, _L, _D), dtype=np.float32)
    for core in range(_NCORES):
        out[core // 4] += res.results[core]["y"]
    return out


# revision 27
# speedup vs baseline: 1.0171x; 1.0171x over previous
"""Causal multi-head attention with RoPE on 8 Trainium2 NeuronCores.

Sharding: core = (batch b, head-group hg): b = core//4, hg = core%4.
Each core computes 4 heads of one batch element end-to-end (QKV projection,
RoPE, causal softmax attention, output-projection partial) and the host sums
the 4 per-head-group partials per batch (the "all-reduce" of the O-proj).

Device-side layout choices (per core):
  xT   [1024, 2048]  x[b] transposed (d on partitions, 8 chunks of 128)
  q^T,k^T  [256, 2048] head-transposed, computed with W^T stationary
  v    [l, e] layout (bf16) with a ones column appended per head
  scores computed transposed: S^T[k, q] = k^T.T @ q^T in f32r (TF32), exp
  with no max subtraction (scores bounded ~17), causal mask applied
  multiplicatively post-exp in bf16, AV matmul in bf16 gives O^T[e+1, q]
  whose last row is the softmax denominator. Reciprocals batched per
  q-chunk on DVE; broadcast across partitions on the idle GpSimd engine.
Score-path matmuls run in float32r (TF32, full PE rate for moving dims
>= 256); the P*V path runs in bf16 (P in [0,1]-scale, errors cancel
between numerator and denominator).
"""

import numpy as np

_B, _L, _D, _H, _HD = 2, 2048, 1024, 16, 64
_HPG = 4              # heads per group (per core)
_EG = _HPG * _HD      # 256
_NCORES = 8
_THETA = 10000.0
_QC = 512             # q-chunk width
_NQC = _L // _QC      # 4
_GK = 2               # k-tiles (128) per exp group
_NKC = _D // 128      # 8 contraction chunks for projections
_LC = 512             # l-chunk for phase 1
_NLC = _L // _LC

_CACHE = {}


def _tf32(a):
    """Round float32 array to TF32 (fp32r): RNE to 10-bit mantissa."""
    b = np.ascontiguousarray(a, dtype=np.float32).view(np.uint32)
    b = (b + np.uint32(0xFFF) + ((b >> np.uint32(13)) & np.uint32(1))) \
        & np.uint32(0xFFFFE000)
    return b.view(np.float32)


def _build_nc(debug_taps=False):
    from contextlib import ExitStack

    import concourse.mybir as mybir
    import concourse.tile as tile
    from concourse import bacc

    f32 = mybir.dt.float32
    f32r = mybir.dt.float32r
    bf16 = mybir.dt.bfloat16
    EXP = mybir.ActivationFunctionType.Exp

    nc = bacc.Bacc("TRN2", target_bir_lowering=False, debug=False,
                   enable_asserts=False)
    xT = nc.dram_tensor("xT", [_D, _L], f32r, kind="ExternalInput")
    wq = nc.dram_tensor("wq", [_D, _EG], f32r, kind="ExternalInput")
    wk = nc.dram_tensor("wk", [_D, _EG], f32r, kind="ExternalInput")
    wv = nc.dram_tensor("wv", [_D, _EG], f32r, kind="ExternalInput")
    wo = nc.dram_tensor("wo", [_EG, _D], f32r, kind="ExternalInput")
    cs = nc.dram_tensor("cs", [128, _L], f32, kind="ExternalInput")
    sn = nc.dram_tensor("sn", [128, _L], f32, kind="ExternalInput")
    msk = nc.dram_tensor("msk", [128, _QC // 128, _QC], bf16,
                         kind="ExternalInput")
    perm = nc.dram_tensor("perm", [128, 128], f32r, kind="ExternalInput")
    vones = nc.dram_tensor("vones", [128, _HD], bf16, kind="ExternalInput")
    onesr = nc.dram_tensor("onesr", [128, _HD], f32r, kind="ExternalInput")
    y = nc.dram_tensor("y", [_L, _D], f32, kind="ExternalOutput")
    taps = {}
    if debug_taps:
        taps["dq"] = nc.dram_tensor("dq", [128, 2, _L], f32r,
                                    kind="ExternalOutput")
        taps["dk"] = nc.dram_tensor("dk", [128, 2, _L], f32r,
                                    kind="ExternalOutput")
        taps["dv"] = nc.dram_tensor("dv", [128, _L // 128, _HPG, _HD + 1],
                                    bf16, kind="ExternalOutput")
        taps["dden"] = nc.dram_tensor("dden", [_NQC, _HPG * _QC], f32,
                                      kind="ExternalOutput")
        taps["drc"] = nc.dram_tensor("drc", [_NQC, _HPG * _QC], f32,
                                     kind="ExternalOutput")
        taps["doT"] = nc.dram_tensor("doT", [128, 2, _L], f32r,
                                     kind="ExternalOutput")
        taps["dpt"] = nc.dram_tensor("dpt", [128, _GK * _QC], bf16,
                                     kind="ExternalOutput")

    with tile.TileContext(nc) as tc, ExitStack() as ctx:
        persist = ctx.enter_context(tc.tile_pool(name="persist", bufs=1))
        qT_sb = persist.tile([128, 2, _L], f32r)
        kT_sb = persist.tile([128, 2, _L], f32r)
        v_sb = persist.tile([128, _L // 128, _HPG, _HD + 4], bf16)
        oT_sb = persist.tile([128, 2, _L], f32r)
        wo_sb = persist.tile([128, 2, _D], f32r)
        msk_sb = persist.tile([128, _QC // 128, _QC], bf16)
        ones_sb = persist.tile([128, _HD], f32r)
        nc.sync.dma_start(out=ones_sb, in_=onesr[:, :])

        # ---- Phase 1: projections + RoPE, streamed over l-chunks ----
        with tc.tile_pool(name="p1", bufs=1) as p1, \
             tc.tile_pool(name="xtp", bufs=2) as xtp, \
             tc.tile_pool(name="p1ps", bufs=3, space="PSUM") as p1ps, \
             tc.tile_pool(name="vps", bufs=2, space="PSUM") as vps, \
             tc.tile_pool(name="rotp", bufs=2, space="PSUM") as rotp, \
             tc.tile_pool(name="rtmp", bufs=3) as rtmp:
            wq_sb = p1.tile([128, _NKC, _EG], f32r)
            wk_sb = p1.tile([128, _NKC, _EG], f32r)
            wv_sb = p1.tile([128, _NKC, _EG], f32r)
            cs_sb = p1.tile([128, _L], f32)
            sn_sb = p1.tile([128, _L], f32)
            perm_sb = p1.tile([128, 128], f32r)
            # split weight loads per contraction chunk so compute can start
            # as soon as the first slices land (startup latency)
            wq_r = wq.rearrange("(c p) e -> p c e", p=128)
            wk_r = wk.rearrange("(c p) e -> p c e", p=128)
            wv_r = wv.rearrange("(c p) e -> p c e", p=128)
            xT_r = xT.rearrange("(c p) l -> p c l", p=128)
            for kc in range(_NKC):
                nc.sync.dma_start(out=wq_sb[:, kc, :], in_=wq_r[:, kc, :])
                nc.sync.dma_start(out=wk_sb[:, kc, :], in_=wk_r[:, kc, :])
            xts = []
            for lc in range(2):
                xt = xtp.tile([128, _NKC, _LC], f32r, tag="xt",
                              name=f"xt{lc}")
                for kc in range(_NKC):
                    nc.sync.dma_start(
                        out=xt[:, kc, :],
                        in_=xT_r[:, kc, lc * _LC:(lc + 1) * _LC])
                xts.append(xt)
            for kc in range(_NKC):
                nc.sync.dma_start(out=wv_sb[:, kc, :], in_=wv_r[:, kc, :])
            nc.sync.dma_start(out=cs_sb, in_=cs[:, :])
            nc.sync.dma_start(out=sn_sb, in_=sn[:, :])
            nc.sync.dma_start(out=perm_sb, in_=perm[:, :])
            nc.sync.dma_start(out=wo_sb,
                              in_=wo.rearrange("(c p) d -> p c d", p=128))
            nc.sync.dma_start(out=msk_sb, in_=msk[:, :, :])
            nc.sync.dma_start(
                out=v_sb[:, :, :, _HD:_HD + 1],
                in_=vones.rearrange("p (a b) -> p a b",
                                    a=_L // 128).unsqueeze(3))

            for lc in range(_NLC):
                ls = slice(lc * _LC, (lc + 1) * _LC)
                if lc < 2:
                    xt = xts[lc]
                else:
                    xt = xtp.tile([128, _NKC, _LC], f32r, tag="xt",
                                  name=f"xt{lc}")
                    for kc in range(_NKC):
                        nc.sync.dma_start(
                            out=xt[:, kc, :],
                            in_=xT_r[:, kc, lc * _LC:(lc + 1) * _LC])
                for w_sb, dst in ((wq_sb, qT_sb), (wk_sb, kT_sb)):
                    for c in range(2):
                        ps = p1ps.tile([128, _LC], f32, tag="proj")
                        for kc in range(_NKC):
                            nc.tensor.matmul(
                                ps, w_sb[:, kc, c * 128:(c + 1) * 128],
                                xt[:, kc, :],
                                start=(kc == 0), stop=(kc == _NKC - 1))
                        nc.vector.tensor_copy(dst[:, c, ls], ps)
                for j in range(_LC // 128):
                    lt = lc * (_LC // 128) + j
                    pv = vps.tile([128, _EG], f32, tag="vp")
                    for kc in range(_NKC):
                        nc.tensor.matmul(
                            pv, xt[:, kc, j * 128:(j + 1) * 128],
                            wv_sb[:, kc, :],
                            start=(kc == 0), stop=(kc == _NKC - 1))
                    nc.vector.tensor_copy(
                        v_sb[:, lt, :, :_HD],
                        pv.rearrange("p (h e) -> p h e", h=_HPG))
                # RoPE (in place on this l-chunk): x*cos + (perm@x)*sin
                for dst in (qT_sb, kT_sb):
                    for c in range(2):
                        rp = rotp.tile([128, _LC], f32, tag="rot")
                        nc.tensor.matmul(rp, perm_sb[:, :], dst[:, c, ls],
                                         start=True, stop=True)
                        tmp = rtmp.tile([128, _LC], f32, tag="rt")
                        nc.vector.tensor_mul(tmp, rp, sn_sb[:, ls])
                        nc.vector.tensor_mul(dst[:, c, ls], dst[:, c, ls],
                                             cs_sb[:, ls])
                        nc.vector.tensor_add(dst[:, c, ls], dst[:, c, ls], tmp)

        if debug_taps:
            nc.sync.dma_start(out=taps["dq"][:, :, :], in_=qT_sb[:, :, :])
            nc.sync.dma_start(out=taps["dk"][:, :, :], in_=kT_sb[:, :, :])
            nc.sync.dma_start(out=taps["dv"][:, :, :, :],
                              in_=v_sb[:, :, :, :_HD + 1])

        # ---- Phase 2: attention + output projection ----
        with tc.tile_pool(name="ptp", bufs=8) as ptp, \
             tc.tile_pool(name="sps", bufs=2, space="PSUM") as sps, \
             tc.tile_pool(name="ops", bufs=2, space="PSUM") as ops, \
             tc.tile_pool(name="opps", bufs=1, space="PSUM") as opps, \
             tc.tile_pool(name="bps", bufs=1, space="PSUM") as bps, \
             tc.tile_pool(name="nrm", bufs=2) as nrm, \
             tc.tile_pool(name="otc", bufs=6) as otcp:
            for qc in range(_NQC):
                q0 = qc * _QC
                qs = slice(q0, q0 + _QC)
                nkt = (qc + 1) * (_QC // 128)
                # denominator rows of all 4 heads, side by side at part. 64
                dsb = nrm.tile([65, _HPG * _QC], f32, tag="dsb")
                otcs = []
                for h in range(_HPG):
                    c, pb = h // 2, 64 * (h % 2)
                    ot = ops.tile([_HD + 1, _QC], f32, tag="ot")
                    ngr = (nkt + _GK - 1) // _GK
                    for g in range(ngr):
                        kts = list(range(g * _GK, min((g + 1) * _GK, nkt)))
                        sp = sps.tile([128, _GK * _QC], f32, tag="sp")
                        for i, kt in enumerate(kts):
                            nc.tensor.matmul(
                                sp[:, i * _QC:(i + 1) * _QC],
                                kT_sb[pb:pb + 64, c, kt * 128:(kt + 1) * 128],
                                qT_sb[pb:pb + 64, c, qs],
                                start=True, stop=True)
                        pt = ptp.tile([128, _GK * _QC], bf16, tag="pt")
                        na = len(kts) * _QC
                        nc.scalar.activation(pt[:, :na], sp[:, :na], EXP,
                                             scale=0.125)
                        if debug_taps and (qc, h, g) == (1, 0, 0):
                            nc.sync.dma_start(out=taps["dpt"][:, :],
                                              in_=pt[:, :])
                        for i, kt in enumerate(kts):
                            dj = kt - qc * (_QC // 128)
                            if dj >= 0:
                                nc.vector.tensor_mul(
                                    pt[:, i * _QC:(i + 1) * _QC],
                                    pt[:, i * _QC:(i + 1) * _QC],
                                    msk_sb[:, dj, :])
                        for i, kt in enumerate(kts):
                            nc.tensor.matmul(
                                ot, v_sb[:, kt, h, :_HD + 1],
                                pt[:, i * _QC:(i + 1) * _QC],
                                start=(kt == 0), stop=(kt == nkt - 1),
                                skip_group_check=True)
                    # free the psum bank quickly: copy numerator+denominator
                    # to SBUF on ACT, stash the den row via SBUF-to-SBUF DMA
                    otc = otcp.tile([_HD + 1, _QC], f32, tag="otc",
                                    name=f"otc{qc}_{h}")
                    nc.scalar.copy(otc, ot[:, :])
                    nc.sync.dma_start(
                        out=dsb[64:65, h * _QC:(h + 1) * _QC],
                        in_=otc[64:65, :])
                    otcs.append(otc)
                # one reciprocal + one fp32r rounding pass per q-chunk
                drec = nrm.tile([65, _HPG * _QC], f32, tag="drec")
                nc.vector.reciprocal(drec[64:65, :], dsb[64:65, :])
                if debug_taps:
                    nc.sync.dma_start(out=taps["dden"][qc:qc + 1, :],
                                      in_=dsb[64:65, :])
                    nc.sync.dma_start(out=taps["drc"][qc:qc + 1, :],
                                      in_=drec[64:65, :])
                drecr = nrm.tile([65, _HPG * _QC], f32r, tag="drecr")
                nc.scalar.copy(drecr[64:65, :], drec[64:65, :])
                for h in range(_HPG):
                    c, pb = h // 2, 64 * (h % 2)
                    # rank-1 broadcast: ones[1,64].T @ recip_row -> [64, 512]
                    bc = bps.tile([64, _QC], f32, tag="bc")
                    nc.tensor.matmul(
                        bc, ones_sb[64:65, :],
                        drecr[64:65, h * _QC:(h + 1) * _QC],
                        start=True, stop=True)
                    otn = otcp.tile([64, _QC], f32r, tag="otn")
                    nc.vector.tensor_mul(otn, otcs[h][0:64, :], bc[:, :])
                    # place normalized O^T at this head's partitions (DMA can
                    # cross partition bases; compute engines cannot)
                    nc.sync.dma_start(out=oT_sb[pb:pb + 64, c, qs], in_=otn)
                # output projection for this q-range
                for j in range(_QC // 128):
                    lt = qc * (_QC // 128) + j
                    for n in range(2):
                        op = opps.tile([128, 512], f32, tag="op")
                        for cc in range(2):
                            nc.tensor.matmul(
                                op, oT_sb[:, cc, lt * 128:(lt + 1) * 128],
                                wo_sb[:, cc, n * 512:(n + 1) * 512],
                                start=(cc == 0), stop=(cc == 1))
                        ob = otcp.tile([128, 512], f32, tag="ob")
                        nc.vector.tensor_copy(ob, op)
                        nc.sync.dma_start(
                            out=y[lt * 128:(lt + 1) * 128,
                                  n * 512:(n + 1) * 512],
                            in_=ob)
            if debug_taps:
                nc.sync.dma_start(out=taps["doT"][:, :, :],
                                  in_=oT_sb[:, :, :])
    nc.compile()
    return nc


def get_nc(debug_taps=False):
    key = ("nc", debug_taps)
    if key not in _CACHE:
        _CACHE[key] = _build_nc(debug_taps)
    return _CACHE[key]


def make_in_maps(x, token_positions, Q, K, V, O_w):
    """Host-side sharding: per-core input dict (core = b*4 + hg)."""
    import ml_dtypes
    bf16 = ml_dtypes.bfloat16
    x = np.asarray(x, dtype=np.float32)
    tp = np.asarray(token_positions)
    Q = np.asarray(Q, dtype=np.float32)
    K = np.asarray(K, dtype=np.float32)
    V = np.asarray(V, dtype=np.float32)
    O_w = np.asarray(O_w, dtype=np.float32)

    # RoPE tables, [128, L]: rows 0..63 head-local e (cos repeated pairwise),
    # rows 64..127 a copy (two heads share one partition tile).
    i = np.arange(_HD // 2, dtype=np.float64)
    denom = _THETA ** (2.0 * i / _HD)                      # [32]
    ang = tp.astype(np.float64)[None, :] / denom[:, None]  # [32, L]
    cs64 = np.repeat(np.cos(ang), 2, axis=0)
    sn64 = np.repeat(np.sin(ang), 2, axis=0)
    cs = np.vstack([cs64, cs64]).astype(np.float32)
    sn = np.vstack([sn64, sn64]).astype(np.float32)

    # pairwise-rotation permutation (rot(x)[2i] = -x[2i+1], rot[2i+1] = x[2i])
    # as a stationary operand: out = permT.T @ x^T = Perm @ x^T
    p64 = np.zeros((64, 64), np.float32)
    for j in range(_HD // 2):
        p64[2 * j + 1, 2 * j] = -1.0
        p64[2 * j, 2 * j + 1] = 1.0
    permT = np.zeros((128, 128), np.float32)
    permT[0:64, 0:64] = p64
    permT[64:128, 64:128] = p64

    # causal masks for the 4 diagonal k-tiles of a 512-wide q-chunk
    pp = np.arange(128)[:, None]
    ff = np.arange(_QC)[None, :]
    msk = np.stack([(ff >= 128 * j + pp) for j in range(_QC // 128)],
                   axis=1).astype(bf16)                   # [128, 4, 512]

    Qr = Q.reshape(_H, _HD, _D)
    Kr = K.reshape(_H, _HD, _D)
    Vr = V.reshape(_H, _HD, _D)

    in_maps = []
    xT = [_tf32(x[b].T) for b in range(_B)]
    for core in range(_NCORES):
        b, hg = core // 4, core % 4
        hs = slice(hg * _HPG, (hg + 1) * _HPG)
        in_maps.append({
            "xT": xT[b],
            "wq": _tf32(Qr[hs].reshape(_EG, _D).T),
            "wk": _tf32(Kr[hs].reshape(_EG, _D).T),
            "wv": _tf32(Vr[hs].reshape(_EG, _D).T),
            "wo": _tf32(O_w[:, hg * _EG:(hg + 1) * _EG].T),
            "cs": cs, "sn": sn, "msk": msk, "perm": permT,
            "vones": np.ones((128, _HD), bf16),
            "onesr": np.ones((128, _HD), np.float32),
        })
    return in_maps


def run_on_hw(in_maps, trace=False, **kw):
    from concourse.bass_utils import run_bass_kernel_spmd
    nc = get_nc()
    return run_bass_kernel_spmd(nc, in_maps, core_ids=list(range(_NCORES)),
                                trace=trace, **kw)


def kernel(x, token_positions, Q, K, V, O_w):
    in_maps = make_in_maps(x, token_positions, Q, K, V, O_w)
    res = run_on_hw(in_maps)
    out = np.zeros((_B, _L, _D), dtype=np.float32)
    for core in range(_NCORES):
        out[core // 4] += res.results[core]["y"]
    return out


# revision 31
# speedup vs baseline: 1.3249x; 1.3025x over previous
"""Causal multi-head attention with RoPE on 8 Trainium2 NeuronCores.

Sharding: core = (batch b, head-group hg): b = core//4, hg = core%4.
Each core computes 4 heads of one batch element end-to-end (QKV projection,
RoPE, causal softmax attention, output-projection partial) and the host sums
the 4 per-head-group partials per batch (the "all-reduce" of the O-proj).

Device-side layout choices (per core):
  xT   [1024, 2048]  x[b] transposed (d on partitions, 8 chunks of 128)
  q^T,k^T  [256, 2048] head-transposed, computed with W^T stationary
  v    [l, e] layout (bf16) with a ones column appended per head
  scores computed transposed: S^T[k, q] = k^T.T @ q^T in f32r (TF32), exp
  with no max subtraction (scores bounded ~17), causal mask applied
  multiplicatively post-exp in bf16, AV matmul in bf16 gives O^T[e+1, q]
  whose last row is the softmax denominator. Reciprocals batched per
  q-chunk on DVE; broadcast across partitions on the idle GpSimd engine.
Score-path matmuls run in float32r (TF32, full PE rate for moving dims
>= 256); the P*V path runs in bf16 (P in [0,1]-scale, errors cancel
between numerator and denominator).
"""

import numpy as np

_B, _L, _D, _H, _HD = 2, 2048, 1024, 16, 64
_HPG = 4              # heads per group (per core)
_EG = _HPG * _HD      # 256
_NCORES = 8
_THETA = 10000.0
_QC = 512             # q-chunk width
_NQC = _L // _QC      # 4
_GK = 2               # k-tiles (128) per exp group
_NKC = _D // 128      # 8 contraction chunks for projections
_LC = 512             # l-chunk for phase 1
_NLC = _L // _LC

_CACHE = {}


def _tf32(a):
    """Round float32 array to TF32 (fp32r): RNE to 10-bit mantissa."""
    b = np.ascontiguousarray(a, dtype=np.float32).view(np.uint32)
    b = (b + np.uint32(0xFFF) + ((b >> np.uint32(13)) & np.uint32(1))) \
        & np.uint32(0xFFFFE000)
    return b.view(np.float32)


def _build_nc(debug_taps=False):
    from contextlib import ExitStack

    import concourse.mybir as mybir
    import concourse.tile as tile
    from concourse import bacc

    f32 = mybir.dt.float32
    f32r = mybir.dt.float32r
    bf16 = mybir.dt.bfloat16
    EXP = mybir.ActivationFunctionType.Exp

    nc = bacc.Bacc("TRN2", target_bir_lowering=False, debug=False,
                   enable_asserts=False)
    xT = nc.dram_tensor("xT", [_D, _L], f32r, kind="ExternalInput")
    wq = nc.dram_tensor("wq", [_D, _EG], f32r, kind="ExternalInput")
    wk = nc.dram_tensor("wk", [_D, _EG], f32r, kind="ExternalInput")
    wv = nc.dram_tensor("wv", [_D, _EG], f32r, kind="ExternalInput")
    wo = nc.dram_tensor("wo", [_EG, _D], f32r, kind="ExternalInput")
    cs = nc.dram_tensor("cs", [128, _L], f32, kind="ExternalInput")
    sn = nc.dram_tensor("sn", [128, _L], f32, kind="ExternalInput")
    msk = nc.dram_tensor("msk", [128, _QC // 128, _QC], bf16,
                         kind="ExternalInput")
    perm = nc.dram_tensor("perm", [128, 128], f32r, kind="ExternalInput")
    vones = nc.dram_tensor("vones", [128, _HD], bf16, kind="ExternalInput")
    onesr = nc.dram_tensor("onesr", [128, _HD], f32r, kind="ExternalInput")
    y = nc.dram_tensor("y", [_L, _D], f32, kind="ExternalOutput")
    taps = {}
    if debug_taps:
        taps["dq"] = nc.dram_tensor("dq", [128, 2, _L], f32r,
                                    kind="ExternalOutput")
        taps["dk"] = nc.dram_tensor("dk", [128, 2, _L], f32r,
                                    kind="ExternalOutput")
        taps["dv"] = nc.dram_tensor("dv", [128, _L // 128, _HPG, _HD + 1],
                                    bf16, kind="ExternalOutput")
        taps["dden"] = nc.dram_tensor("dden", [_NQC, _HPG * _QC], f32,
                                      kind="ExternalOutput")
        taps["drc"] = nc.dram_tensor("drc", [_NQC, _HPG * _QC], f32,
                                     kind="ExternalOutput")
        taps["doT"] = nc.dram_tensor("doT", [128, 2, _L], f32r,
                                     kind="ExternalOutput")
        taps["dpt"] = nc.dram_tensor("dpt", [128, _GK * _QC], bf16,
                                     kind="ExternalOutput")

    with tile.TileContext(nc) as tc, ExitStack() as ctx:
        persist = ctx.enter_context(tc.tile_pool(name="persist", bufs=1))
        qT_sb = persist.tile([128, 2, _L], f32r)
        kT_sb = persist.tile([128, 2, _L], f32r)
        v_sb = persist.tile([128, _L // 128, _HPG, _HD + 4], bf16)
        oT_sb = persist.tile([128, 2, _L], f32r)
        wo_sb = persist.tile([128, 2, _D], f32r)
        msk_sb = persist.tile([128, _QC // 128, _QC], bf16)
        ones_sb = persist.tile([128, _HD], f32r)
        nc.sync.dma_start(out=ones_sb, in_=onesr[:, :])

        # ---- Phase 1: projections + RoPE, streamed over l-chunks ----
        with tc.tile_pool(name="p1", bufs=1) as p1, \
             tc.tile_pool(name="xtp", bufs=2) as xtp, \
             tc.tile_pool(name="p1ps", bufs=3, space="PSUM") as p1ps, \
             tc.tile_pool(name="vps", bufs=2, space="PSUM") as vps, \
             tc.tile_pool(name="rotp", bufs=2, space="PSUM") as rotp, \
             tc.tile_pool(name="rtmp", bufs=3) as rtmp:
            wq_sb = p1.tile([128, _NKC, _EG], f32r)
            wk_sb = p1.tile([128, _NKC, _EG], f32r)
            wv_sb = p1.tile([128, _NKC, _EG], f32r)
            cs_sb = p1.tile([128, _L], f32)
            sn_sb = p1.tile([128, _L], f32)
            perm_sb = p1.tile([128, 128], f32r)
            # split weight loads per contraction chunk so compute can start
            # as soon as the first slices land (startup latency)
            wq_r = wq.rearrange("(c p) e -> p c e", p=128)
            wk_r = wk.rearrange("(c p) e -> p c e", p=128)
            wv_r = wv.rearrange("(c p) e -> p c e", p=128)
            xT_r = xT.rearrange("(c p) l -> p c l", p=128)
            for kc in range(_NKC):
                nc.sync.dma_start(out=wq_sb[:, kc, :], in_=wq_r[:, kc, :])
                nc.sync.dma_start(out=wk_sb[:, kc, :], in_=wk_r[:, kc, :])
            xts = []
            for lc in range(2):
                xt = xtp.tile([128, _NKC, _LC], f32r, tag="xt",
                              name=f"xt{lc}")
                for kc in range(_NKC):
                    nc.sync.dma_start(
                        out=xt[:, kc, :],
                        in_=xT_r[:, kc, lc * _LC:(lc + 1) * _LC])
                xts.append(xt)
            for kc in range(_NKC):
                nc.sync.dma_start(out=wv_sb[:, kc, :], in_=wv_r[:, kc, :])
            nc.sync.dma_start(out=cs_sb, in_=cs[:, :])
            nc.sync.dma_start(out=sn_sb, in_=sn[:, :])
            nc.sync.dma_start(out=perm_sb, in_=perm[:, :])
            nc.sync.dma_start(out=wo_sb,
                              in_=wo.rearrange("(c p) d -> p c d", p=128))
            nc.sync.dma_start(out=msk_sb, in_=msk[:, :, :])
            nc.sync.dma_start(
                out=v_sb[:, :, :, _HD:_HD + 1],
                in_=vones.rearrange("p (a b) -> p a b",
                                    a=_L // 128).unsqueeze(3))

            for lc in range(_NLC):
                ls = slice(lc * _LC, (lc + 1) * _LC)
                if lc < 2:
                    xt = xts[lc]
                else:
                    xt = xtp.tile([128, _NKC, _LC], f32r, tag="xt",
                                  name=f"xt{lc}")
                    for kc in range(_NKC):
                        nc.sync.dma_start(
                            out=xt[:, kc, :],
                            in_=xT_r[:, kc, lc * _LC:(lc + 1) * _LC])
                for w_sb, dst in ((wq_sb, qT_sb), (wk_sb, kT_sb)):
                    for c in range(2):
                        ps = p1ps.tile([128, _LC], f32, tag="proj")
                        for kc in range(_NKC):
                            nc.tensor.matmul(
                                ps, w_sb[:, kc, c * 128:(c + 1) * 128],
                                xt[:, kc, :],
                                start=(kc == 0), stop=(kc == _NKC - 1))
                        nc.vector.tensor_copy(dst[:, c, ls], ps)
                for j in range(_LC // 128):
                    lt = lc * (_LC // 128) + j
                    pv = vps.tile([128, _EG], f32, tag="vp")
                    for kc in range(_NKC):
                        nc.tensor.matmul(
                            pv, xt[:, kc, j * 128:(j + 1) * 128],
                            wv_sb[:, kc, :],
                            start=(kc == 0), stop=(kc == _NKC - 1))
                    nc.vector.tensor_copy(
                        v_sb[:, lt, :, :_HD],
                        pv.rearrange("p (h e) -> p h e", h=_HPG))
                # RoPE (in place on this l-chunk): x*cos + (perm@x)*sin
                for dst in (qT_sb, kT_sb):
                    for c in range(2):
                        rp = rotp.tile([128, _LC], f32, tag="rot")
                        nc.tensor.matmul(rp, perm_sb[:, :], dst[:, c, ls],
                                         start=True, stop=True)
                        tmp = rtmp.tile([128, _LC], f32, tag="rt")
                        nc.vector.tensor_mul(tmp, rp, sn_sb[:, ls])
                        nc.vector.tensor_mul(dst[:, c, ls], dst[:, c, ls],
                                             cs_sb[:, ls])
                        nc.vector.tensor_add(dst[:, c, ls], dst[:, c, ls], tmp)

        if debug_taps:
            nc.sync.dma_start(out=taps["dq"][:, :, :], in_=qT_sb[:, :, :])
            nc.sync.dma_start(out=taps["dk"][:, :, :], in_=kT_sb[:, :, :])
            nc.sync.dma_start(out=taps["dv"][:, :, :, :],
                              in_=v_sb[:, :, :, :_HD + 1])

        # ---- Phase 2: attention + output projection ----
        # Normalization + O-proj of q-chunk N are deferred into the attention
        # stream of q-chunk N+1 so the PE never waits on the (DVE/ACT/DMA)
        # normalization chain: by then the reciprocals are long done.
        with tc.tile_pool(name="ptp", bufs=8) as ptp, \
             tc.tile_pool(name="sps", bufs=2, space="PSUM") as sps, \
             tc.tile_pool(name="ops", bufs=2, space="PSUM") as ops, \
             tc.tile_pool(name="opps", bufs=2, space="PSUM") as opps, \
             tc.tile_pool(name="nrm", bufs=2) as nrm, \
             tc.tile_pool(name="otc", bufs=10) as otcp:

            def norm_head(st, h):
                """broadcast recip + normalize + place head h of chunk st."""
                qc, qs, otcs, drow = st["qc"], st["qs"], st["otcs"], st["drow"]
                c, pb = h // 2, 64 * (h % 2)
                # rank-1 broadcast: ones[1,64].T @ recip_row -> [64, 512]
                bc = opps.tile([128, _QC], f32, tag="op", name=f"bc{qc}_{h}")
                bc = bc[0:64, :]
                nc.tensor.matmul(
                    bc, ones_sb[64:65, :],
                    drow[64:65, h * _QC:(h + 1) * _QC],
                    start=True, stop=True)
                otn = otcp.tile([64, _QC], f32r, tag="otn",
                                name=f"otn{qc}_{h}")
                nc.vector.tensor_mul(otn, otcs[h][0:64, :], bc[:, :])
                # place normalized O^T at this head's partitions (DMA can
                # cross partition bases; compute engines cannot)
                nc.sync.dma_start(out=oT_sb[pb:pb + 64, c, qs], in_=otn)

            def oproj_tile(st, j):
                """output projection for l-tile j of chunk st."""
                qc = st["qc"]
                lt = qc * (_QC // 128) + j
                for n in range(2):
                    op = opps.tile([128, 512], f32, tag="op",
                                   name=f"op{qc}_{j}_{n}")
                    for cc in range(2):
                        nc.tensor.matmul(
                            op, oT_sb[:, cc, lt * 128:(lt + 1) * 128],
                            wo_sb[:, cc, n * 512:(n + 1) * 512],
                            start=(cc == 0), stop=(cc == 1))
                    ob = otcp.tile([128, 512], f32, tag="ob")
                    nc.vector.tensor_copy(ob, op)
                    nc.sync.dma_start(
                        out=y[lt * 128:(lt + 1) * 128,
                              n * 512:(n + 1) * 512],
                        in_=ob)

            prev = None
            for qc in range(_NQC):
                q0 = qc * _QC
                qs = slice(q0, q0 + _QC)
                nkt = (qc + 1) * (_QC // 128)
                # denominators land transposed [128, 4] per head so the
                # reciprocal is partition-parallel (free size 16, not 2048)
                dsb = nrm.tile([128, _HPG * 4], f32, tag="dsb")
                otcs = []
                for h in range(_HPG):
                    c, pb = h // 2, 64 * (h % 2)
                    ot = ops.tile([_HD + 1, _QC], f32, tag="ot")
                    ngr = (nkt + _GK - 1) // _GK
                    for g in range(ngr):
                        kts = list(range(g * _GK, min((g + 1) * _GK, nkt)))
                        sp = sps.tile([128, _GK * _QC], f32, tag="sp")
                        for i, kt in enumerate(kts):
                            nc.tensor.matmul(
                                sp[:, i * _QC:(i + 1) * _QC],
                                kT_sb[pb:pb + 64, c, kt * 128:(kt + 1) * 128],
                                qT_sb[pb:pb + 64, c, qs],
                                start=True, stop=True)
                        pt = ptp.tile([128, _GK * _QC], bf16, tag="pt")
                        na = len(kts) * _QC
                        nc.scalar.activation(pt[:, :na], sp[:, :na], EXP,
                                             scale=0.125)
                        if debug_taps and (qc, h, g) == (1, 0, 0):
                            nc.sync.dma_start(out=taps["dpt"][:, :],
                                              in_=pt[:, :])
                        for i, kt in enumerate(kts):
                            dj = kt - qc * (_QC // 128)
                            if dj >= 0:
                                nc.vector.tensor_mul(
                                    pt[:, i * _QC:(i + 1) * _QC],
                                    pt[:, i * _QC:(i + 1) * _QC],
                                    msk_sb[:, dj, :])
                        for i, kt in enumerate(kts):
                            nc.tensor.matmul(
                                ot, v_sb[:, kt, h, :_HD + 1],
                                pt[:, i * _QC:(i + 1) * _QC],
                                start=(kt == 0), stop=(kt == nkt - 1),
                                skip_group_check=True)
                    # free the psum bank quickly: copy numerator+denominator
                    # to SBUF on ACT, stash the den row (transposed to
                    # [128, 4]) via SBUF-to-SBUF DMA
                    otc = otcp.tile([_HD + 1, _QC], f32, tag="otc",
                                    name=f"otc{qc}_{h}")
                    nc.scalar.copy(otc, ot[:, :])
                    nc.sync.dma_start(
                        out=dsb[:, h * 4:(h + 1) * 4],
                        in_=otc[64:65, :])
                    otcs.append(otc)
                    # interleave deferred work of the previous q-chunk to
                    # keep the PE stream dense while ACT runs the exps
                    if prev is not None:
                        if h == 0:
                            norm_head(prev, 0)
                            norm_head(prev, 1)
                        elif h == 1:
                            norm_head(prev, 2)
                            norm_head(prev, 3)
                            oproj_tile(prev, 0)
                        elif h == 2:
                            oproj_tile(prev, 1)
                            oproj_tile(prev, 2)
                        else:
                            oproj_tile(prev, 3)
                # reciprocal (partition-parallel) + fp32r rounding + row-
                # layout restore for the broadcast matmuls
                drec = nrm.tile([128, _HPG * 4], f32, tag="drec")
                nc.vector.reciprocal(drec, dsb)
                drecr = nrm.tile([128, _HPG * 4], f32r, tag="drecr")
                nc.scalar.copy(drecr, drec)
                drow = nrm.tile([65, _HPG * _QC], f32r, tag="drow")
                for h in range(_HPG):
                    nc.sync.dma_start(
                        out=drow[64:65, h * _QC:(h + 1) * _QC],
                        in_=drecr[:, h * 4:(h + 1) * 4])
                if debug_taps:
                    nc.sync.dma_start(out=taps["dden"][qc:qc + 1, :],
                                      in_=dsb[:, :])
                    nc.sync.dma_start(out=taps["drc"][qc:qc + 1, :],
                                      in_=drow[64:65, :])
                prev = {"qc": qc, "qs": qs, "otcs": otcs, "drow": drow}
            # tail: last q-chunk's normalization + projection
            for hh in range(_HPG):
                norm_head(prev, hh)
            for j in range(_QC // 128):
                oproj_tile(prev, j)
            if debug_taps:
                nc.sync.dma_start(out=taps["doT"][:, :, :],
                                  in_=oT_sb[:, :, :])
    nc.compile()
    return nc


def get_nc(debug_taps=False):
    key = ("nc", debug_taps)
    if key not in _CACHE:
        _CACHE[key] = _build_nc(debug_taps)
    return _CACHE[key]


def make_in_maps(x, token_positions, Q, K, V, O_w):
    """Host-side sharding: per-core input dict (core = b*4 + hg)."""
    import ml_dtypes
    bf16 = ml_dtypes.bfloat16
    x = np.asarray(x, dtype=np.float32)
    tp = np.asarray(token_positions)
    Q = np.asarray(Q, dtype=np.float32)
    K = np.asarray(K, dtype=np.float32)
    V = np.asarray(V, dtype=np.float32)
    O_w = np.asarray(O_w, dtype=np.float32)

    # RoPE tables, [128, L]: rows 0..63 head-local e (cos repeated pairwise),
    # rows 64..127 a copy (two heads share one partition tile).
    i = np.arange(_HD // 2, dtype=np.float64)
    denom = _THETA ** (2.0 * i / _HD)                      # [32]
    ang = tp.astype(np.float64)[None, :] / denom[:, None]  # [32, L]
    cs64 = np.repeat(np.cos(ang), 2, axis=0)
    sn64 = np.repeat(np.sin(ang), 2, axis=0)
    cs = np.vstack([cs64, cs64]).astype(np.float32)
    sn = np.vstack([sn64, sn64]).astype(np.float32)

    # pairwise-rotation permutation (rot(x)[2i] = -x[2i+1], rot[2i+1] = x[2i])
    # as a stationary operand: out = permT.T @ x^T = Perm @ x^T
    p64 = np.zeros((64, 64), np.float32)
    for j in range(_HD // 2):
        p64[2 * j + 1, 2 * j] = -1.0
        p64[2 * j, 2 * j + 1] = 1.0
    permT = np.zeros((128, 128), np.float32)
    permT[0:64, 0:64] = p64
    permT[64:128, 64:128] = p64

    # causal masks for the 4 diagonal k-tiles of a 512-wide q-chunk
    pp = np.arange(128)[:, None]
    ff = np.arange(_QC)[None, :]
    msk = np.stack([(ff >= 128 * j + pp) for j in range(_QC // 128)],
                   axis=1).astype(bf16)                   # [128, 4, 512]

    Qr = Q.reshape(_H, _HD, _D)
    Kr = K.reshape(_H, _HD, _D)
    Vr = V.reshape(_H, _HD, _D)

    in_maps = []
    xT = [_tf32(x[b].T) for b in range(_B)]
    for core in range(_NCORES):
        b, hg = core // 4, core % 4
        hs = slice(hg * _HPG, (hg + 1) * _HPG)
        in_maps.append({
            "xT": xT[b],
            "wq": _tf32(Qr[hs].reshape(_EG, _D).T),
            "wk": _tf32(Kr[hs].reshape(_EG, _D).T),
            "wv": _tf32(Vr[hs].reshape(_EG, _D).T),
            "wo": _tf32(O_w[:, hg * _EG:(hg + 1) * _EG].T),
            "cs": cs, "sn": sn, "msk": msk, "perm": permT,
            "vones": np.ones((128, _HD), bf16),
            "onesr": np.ones((128, _HD), np.float32),
        })
    return in_maps


def run_on_hw(in_maps, trace=False, **kw):
    from concourse.bass_utils import run_bass_kernel_spmd
    nc = get_nc()
    return run_bass_kernel_spmd(nc, in_maps, core_ids=list(range(_NCORES)),
                                trace=trace, **kw)


def kernel(x, token_positions, Q, K, V, O_w):
    in_maps = make_in_maps(x, token_positions, Q, K, V, O_w)
    res = run_on_hw(in_maps)
    out = np.zeros((_B, _L, _D), dtype=np.float32)
    for core in range(_NCORES):
        out[core // 4] += res.results[core]["y"]
    return out
